# revision 4
# baseline (speedup 1.0000x reference)
"""FPS (farthest point sampling) Trainium2 kernel, v2.

Problem: x (64, 65536, 3) fp32 -> y (64, 2048, 3): per cloud, iteratively
select the point maximizing min-distance-to-selected-set, starting at index 0
(exact argmax semantics incl. first-index tie-breaks).

Sharding: data-parallel over batch. 8 clouds per core; 2 groups of 4 clouds,
each group = [128 partitions x 2048 free] planes (cloud = 32 partitions).

Per FPS iteration (all on-chip), per group:
  ACT : dx2/dy2/dz2 = Square(-x + p)  (scale=-1, bias=+coords)   3 passes
  GpS : s1 = dx2 + dy2                                           1 pass
  DVE+GpS : s = s1 + dz2  (split by free-dim columns)            1 pass
  DVE : md = min(md, s); pm8 = max8(md); idx8 = max_index        3 passes
  tail: ACT enc encode -> PE transpose pm/enc -> DVE small argmax-over-
        partitions w/ first-index tiebreak -> ACT row cast -> indirect
        gather winner coords -> PE broadcast to npc -> direct DMA y out.
"""
import sys
import types
import numpy as np

B, N, M = 64, 65536, 2048
NCORES = 8
BPC = B // NCORES          # clouds per core = 8
NGROUPS = 2
CPG = BPC // NGROUPS       # clouds per group = 4
PP = 128 // CPG            # partitions per cloud = 32
FD = N // PP               # free dim per partition = 2048
BIGK = float(1 << 24)
FLT_MAX = 3.4028235e38
ADD2_DVE = 896             # add2 columns computed on DVE (rest on GpSimd)

_cached = {}


def _install_compat():
    """Environment workarounds: NTFF hook shim + 1-sync-wait-per-instruction
    splitter for this walrus build."""
    try:
        from antenv import axon_hooks  # noqa: F401
    except ImportError:
        try:
            from trn_agent_boot.trn_boot import _ntff_profile_via_ctypes
            _hook = _ntff_profile_via_ctypes('/opt/axon/libaxon_pjrt.so')
        except Exception:
            _hook = None
        _mod = types.ModuleType("antenv.axon_hooks")
        _mod.get_axon_ntff_profile_hook = lambda: _hook
        _mod.set_axon_ntff_profile_hook = lambda h: None
        sys.modules['antenv.axon_hooks'] = _mod

    import concourse.tile as tile_mod
    import concourse.mybir as mybir
    from bass_rust import ScopedClock
    import bass_rust as _br

    if getattr(tile_mod.TileContext, "_fps_patched", False):
        return
    tile_mod.TileContext._fps_patched = True

    _orig_lower = tile_mod.TileContext._lower_ordered_insts

    def _split_waits(self, ordered):
        sem_ids = {}
        try:
            for nm, h in self.sems.allocated().items():
                sem_ids[getattr(h, "name", nm)] = h.num
        except Exception:
            pass
        for bb_name, insts in ordered.items():
            out = []
            for inst in insts:
                si = inst.sync_info
                if type(inst).__name__ == "InstIncSwdgeSem":
                    names = inst._sem_names
                    vals = inst._sem_values
                    mode = str(inst._mode)
                    sgn = -1 if "sub" in mode else 1
                    waits = {w.ant_name: w for w in (
                        list(si.on_wait) if si is not None else [])}
                    for nm, v in zip(names, vals):
                        upd = _br.SyncUpdate(
                            sync_type='semaphore', id=sem_ids[nm],
                            ant_name=nm, update_mode='sem-inc',
                            update_value=sgn * v, update_reg=None)
                        w = waits.pop(nm, None)
                        nop = mybir.InstNoOp(
                            name=self.nc.get_next_instruction_name(),
                            engine=inst.engine,
                            sync_info=mybir.SyncInfo(
                                on_wait=[w] if w is not None else [],
                                on_update=[upd]),
                            bass_nofuse=True,
                        )
                        out.append(nop)
                    for w in waits.values():
                        nop = mybir.InstNoOp(
                            name=self.nc.get_next_instruction_name(),
                            engine=inst.engine,
                            sync_info=mybir.SyncInfo(on_wait=[w], on_update=[]),
                            bass_nofuse=True,
                        )
                        out.append(nop)
                    continue
                if si is not None and len(si.on_wait) > 1:
                    waits = list(si.on_wait)
                    for w in waits[:-1]:
                        nop = mybir.InstNoOp(
                            name=self.nc.get_next_instruction_name(),
                            engine=inst.engine,
                            sync_info=mybir.SyncInfo(on_wait=[w], on_update=[]),
                            bass_nofuse=True,
                        )
                        out.append(nop)
                    si.on_wait = waits[-1:]
                    inst.sync_info = si
                out.append(inst)
            insts[:] = out
        return _orig_lower(self, ordered)

    tile_mod.TileContext._lower_ordered_insts = _split_waits

    def _patched_drain_and_barrier(self, tick_clock, wait_clock):
        probe = self.nc.sync.nop(nofuse=True)
        wait_clock.add_sem_waits(
            probe.ins, ScopedClock({None: tick_clock.global_clock})
        )
        si = probe.ins.sync_info
        waits = list(si.on_wait)
        if len(waits) > 1:
            si.on_wait = waits[:1]
            probe.ins.sync_info = si
            for w in waits[1:]:
                extra = self.nc.sync.nop(nofuse=True)
                extra.ins.sync_info = _br.SyncInfo(on_wait=[w], on_update=[])
        self.nc.sync.drain()
        self.nc.all_engine_barrier()
        assert self.sems is not None
        popped = self.nc._tile_sem_poison_stack.pop()
        assert popped is self._sem_poison
        sems = list(self.sems.allocated().values())
        if sems:
            sem_nums = [getattr(s_, "num", s_) for s_ in sems]
            self.nc._state.prepend_free_semaphores(sem_nums)
            for poison_set in self.nc._tile_sem_poison_stack:
                poison_set.update(sem_nums)
        self.nc.all_engine_barrier()

    tile_mod.TileContext._drain_and_barrier = _patched_drain_and_barrier


def _build(n_iters=M):
    import concourse.bass as bass
    import concourse.mybir as mybir
    from concourse.tile import TileContext
    from concourse.bass import IndirectOffsetOnAxis

    fp = mybir.dt.float32
    i32 = mybir.dt.int32
    u32 = mybir.dt.uint32
    Act = mybir.ActivationFunctionType
    Alu = mybir.AluOpType
    nc = bass.Bass("TRN2", target_bir_lowering=False, debug=False)

    x_d = nc.dram_tensor("x", [BPC * N, 3], fp, kind="ExternalInput")
    y_d = nc.dram_tensor("out", [BPC * M, 3], fp, kind="ExternalOutput")
    ident_d = nc.dram_tensor("ident", [128, 128], fp, kind="ExternalInput")
    memb01_d = nc.dram_tensor("memb01", [CPG, 128], fp, kind="ExternalInput")
    pbase_d = nc.dram_tensor("pbase", [128, 1], fp, kind="ExternalInput")
    kcg_d = nc.dram_tensor("kcg", [NGROUPS * CPG, 1], fp, kind="ExternalInput")
    rows0_d = nc.dram_tensor("rows0", [NGROUPS * CPG, 1], i32,
                             kind="ExternalInput")

    y3 = y_d.ap().rearrange("(b m) c -> b m c", m=M)

    with TileContext(nc) as tc:
        import contextlib
        with contextlib.ExitStack() as ctx:
            cpool = ctx.enter_context(tc.tile_pool(name="consts", bufs=1))
            ident = cpool.tile([128, 128], fp, tag="ident")
            nc.sync.dma_start(ident[:, :], ident_d[:, :])
            memb01 = cpool.tile([CPG, 128], fp, tag="memb01")
            nc.sync.dma_start(memb01[:, :], memb01_d[:, :])
            pbase = cpool.tile([128, 1], fp, tag="pbase")
            nc.sync.dma_start(pbase[:, :], pbase_d[:, :])

            G = []
            for g in range(NGROUPS):
                gp = ctx.enter_context(tc.tile_pool(name=f"g{g}", bufs=1))
                pg = ctx.enter_context(
                    tc.tile_pool(name=f"p{g}", bufs=1, space="PSUM"))
                st = {}
                for nm in ("xs", "ys", "zs", "md", "dx2", "dy2", "dz2"):
                    st[nm] = gp.tile([128, FD], fp, tag=nm, name=f"{nm}_{g}")
                # tb: cols 0:8 = max8 out, col 8 = enc encode
                st["tb"] = gp.tile([128, 9], fp, tag="tb", name=f"tb_{g}")
                st["idx8"] = gp.tile([128, 8], u32, tag="idx8",
                                     name=f"idx8_{g}")
                st["gm4"] = gp.tile([1, CPG], fp, tag="gm4", name=f"gm4_{g}")
                st["wA"] = gp.tile([1, 128], fp, tag="wA", name=f"wA_{g}")
                st["wB"] = gp.tile([1, 128], fp, tag="wB", name=f"wB_{g}")
                st["enc4n"] = gp.tile([1, CPG], fp, tag="enc4n",
                                      name=f"enc4n_{g}")
                st["kcg4"] = gp.tile([CPG, 1], fp, tag="kcg4",
                                     name=f"kcg4_{g}")
                st["rows4"] = gp.tile([CPG, 1], i32, tag="rows4",
                                      name=f"rows4_{g}")
                st["pts"] = gp.tile([CPG, 3], fp, tag="pts", name=f"pts_{g}")
                st["npc"] = gp.tile([128, 3], fp, tag="npc", name=f"npc_{g}")
                st["ps_t"] = pg.tile([1, 256], fp, tag=f"ps_t{g}",
                                     name=f"ps_t_{g}")
                st["ps_r"] = pg.tile([CPG, 1], fp, tag=f"ps_r{g}",
                                     name=f"ps_r_{g}")
                st["ps_c"] = pg.tile([128, 3], fp, tag=f"ps_c{g}",
                                     name=f"ps_c_{g}")
                G.append(st)

                nc.sync.dma_start(
                    st["kcg4"][:, :], kcg_d[g * CPG:(g + 1) * CPG, :])

                # load x for this group, split into coordinate planes
                xall = gp.tile([128, FD * 3], fp, tag="xall", name=f"xall_{g}")
                xv2 = x_d.ap().rearrange("(p f) c -> p (f c)", f=FD)
                base = g * CPG * PP
                for sl in range(0, 128, 16):
                    nc.sync.dma_start(
                        xall[sl:sl + 16, :],
                        xv2[base + sl:base + sl + 16, :])
                x3 = xall[:, :].rearrange("p (f c) -> p f c", c=3)
                for nm, c in (("xs", 0), ("ys", 1), ("zs", 2)):
                    nc.vector.tensor_copy(st[nm][:, :], x3[:, :, c])
                nc.vector.memset(st["md"][:, :], FLT_MAX)

                # bootstrap: winner of "iteration 0" is point 0 of each cloud
                nc.sync.dma_start(
                    st["rows4"][:, :], rows0_d[g * CPG:(g + 1) * CPG, :])
                nc.gpsimd.indirect_dma_start(
                    out=st["pts"][:, :], out_offset=None,
                    in_=x_d[:, :],
                    in_offset=IndirectOffsetOnAxis(ap=st["rows4"][:, :],
                                                   axis=0),
                )
                nc.sync.dma_start(
                    y3[g * CPG:(g + 1) * CPG, 0:1, :],
                    st["pts"][:, :].unsqueeze(1))
                nc.tensor.matmul(
                    st["ps_c"][:, :], memb01[:, :], st["pts"][:, :])
                nc.scalar.copy(st["npc"][:, :], st["ps_c"][:, :])

            def emit_big(g):
                st = G[g]
                npc = st["npc"]
                # squares: (p - x)^2 via scale=-1, bias=+p
                nc.scalar.activation(st["dx2"][:, :], st["xs"][:, :],
                                     Act.Square, bias=npc[:, 0:1], scale=-1.0)
                nc.scalar.activation(st["dy2"][:, :], st["ys"][:, :],
                                     Act.Square, bias=npc[:, 1:2], scale=-1.0)
                nc.scalar.activation(st["dz2"][:, :], st["zs"][:, :],
                                     Act.Square, bias=npc[:, 2:3], scale=-1.0)
                # s = (dx2 + dy2) + dz2 via CCE accum DMAs (keeps the adds
                # entirely off the compute engines); dx2 is the accumulator.
                # Two independent half-column chains to halve the latency.
                H = FD // 2
                for lo, hi in ((0, H), (H, FD)):
                    nc.gpsimd.dma_start(st["dx2"][:, lo:hi],
                                        st["dy2"][:, lo:hi],
                                        accum_op=Alu.add)
                for lo, hi in ((0, H), (H, FD)):
                    nc.gpsimd.dma_start(st["dx2"][:, lo:hi],
                                        st["dz2"][:, lo:hi],
                                        accum_op=Alu.add)
                # md = min(md, s)
                nc.vector.tensor_tensor(
                    out=st["md"][:, :], in0=st["md"][:, :],
                    in1=st["dx2"][:, :], op=Alu.min)
                # per-partition top-8 + first-index of max
                nc.vector.max(out=st["tb"][:, 0:8], in_=st["md"][:, :])
                nc.vector.max_index(
                    out=st["idx8"][:, :], in_max=st["tb"][:, 0:8],
                    in_values=st["md"][:, :])

            def emit_tail(g, t):
                st = G[g]
                # enc = pbase - idx  (DVE tensor_scalar, 2x mode)
                nc.vector.tensor_scalar(
                    out=st["tb"][:, 8:9], in0=st["idx8"][:, 0:1],
                    scalar1=-1.0, scalar2=pbase[:, 0:1],
                    op0=Alu.mult, op1=Alu.add)
                # transpose pm and enc into one PSUM row
                nc.tensor.transpose(
                    st["ps_t"][0:1, 0:128], st["tb"][:, 0:1], ident[:, :])
                nc.tensor.transpose(
                    st["ps_t"][0:1, 128:256], st["tb"][:, 8:9], ident[:, :])
                pv = st["ps_t"][0:1, 0:128].rearrange(
                    "o (c p) -> o c p", p=PP)
                ev = st["ps_t"][0:1, 128:256].rearrange(
                    "o (c p) -> o c p", p=PP)
                # per-cloud max pm
                nc.vector.tensor_reduce(
                    out=st["gm4"][0:1, :], in_=pv,
                    axis=mybir.AxisListType.X, op=Alu.max)
                # winner mask & lexicographic argmax via enc
                gbc = st["gm4"][0:1, :].unsqueeze(2).broadcast_to(
                    [1, CPG, PP])
                nc.vector.tensor_tensor(
                    out=st["wA"][0:1, :].rearrange("o (c p) -> o c p", p=PP),
                    in0=pv, in1=gbc, op=Alu.is_ge)
                nc.vector.tensor_tensor(
                    out=st["wB"][0:1, :], in0=st["wA"][0:1, :],
                    in1=st["ps_t"][0:1, 128:256], op=Alu.mult)
                # enc4n = -max(enc masked) per cloud
                nc.vector.tensor_reduce(
                    out=st["enc4n"][0:1, :],
                    in_=st["wB"][0:1, :].rearrange("o (c p) -> o c p", p=PP),
                    axis=mybir.AxisListType.X, op=Alu.max, negate=True)
                # to partitions: ps_r[c,0] = enc4n[0,c]
                nc.tensor.matmul(
                    st["ps_r"][:, :], st["enc4n"][0:1, :], ident[0:1, 0:1])
                # rows = kcg + (-enc)  (cast to int32, DVE)
                nc.vector.tensor_scalar(
                    out=st["rows4"][:, :], in0=st["ps_r"][:, :],
                    scalar1=st["kcg4"][:, 0:1], scalar2=None, op0=Alu.add)
                # gather winner coords; write y; broadcast +coords to npc
                nc.gpsimd.indirect_dma_start(
                    out=st["pts"][:, :], out_offset=None,
                    in_=x_d[:, :],
                    in_offset=IndirectOffsetOnAxis(ap=st["rows4"][:, :],
                                                   axis=0),
                )
                nc.sync.dma_start(
                    y3[g * CPG:(g + 1) * CPG, t:t + 1, :],
                    st["pts"][:, :].unsqueeze(1))

            def emit_npc(g, t):
                st = G[g]
                if t < n_iters - 1:
                    nc.tensor.matmul(
                        st["ps_c"][:, :], memb01[:, :], st["pts"][:, :])
                    nc.scalar.copy(st["npc"][:, :], st["ps_c"][:, :])

            # DVE-relay emission: groups alternate through the DVE queue
            for t in range(1, n_iters):
                emit_big(0)
                emit_tail(0, t)
                emit_big(1)
                emit_tail(1, t)
                emit_npc(0, t)
                emit_npc(1, t)
    return nc


def _host_consts():
    ident = np.eye(128, dtype=np.float32)
    memb01 = np.zeros((CPG, 128), dtype=np.float32)
    for c in range(CPG):
        memb01[c, c * PP:(c + 1) * PP] = 1.0
    pbase = (BIGK - np.arange(128, dtype=np.float64) * FD).astype(
        np.float32).reshape(128, 1)
    kcg = np.zeros((NGROUPS * CPG, 1), dtype=np.float32)
    for g in range(NGROUPS):
        kcg[g * CPG:(g + 1) * CPG, 0] = BIGK + g * CPG * N
    rows0 = (np.arange(BPC, dtype=np.int32) * N).reshape(NGROUPS * CPG, 1)
    return ident, memb01, pbase, kcg, rows0


def kernel(x: np.ndarray) -> np.ndarray:
    _install_compat()
    from concourse.bass_utils import run_bass_kernel_spmd

    if "nc" not in _cached:
        _cached["nc"] = _build()
    nc = _cached["nc"]

    ident, memb01, pbase, kcg, rows0 = _host_consts()
    x = np.ascontiguousarray(x, dtype=np.float32)
    in_maps = []
    for core in range(NCORES):
        shard = x[core * BPC:(core + 1) * BPC].reshape(BPC * N, 3)
        in_maps.append({
            "x": shard, "ident": ident, "memb01": memb01,
            "pbase": pbase, "kcg": kcg, "rows0": rows0,
        })
    res = run_bass_kernel_spmd(nc, in_maps, core_ids=list(range(NCORES)))
    out = np.concatenate(
        [res.results[i]["out"].reshape(BPC, M, 3) for i in range(NCORES)],
        axis=0)
    return out.astype(np.float32)
